# revision 8
# baseline (speedup 1.0000x reference)
"""Trainium2 kernel for nn_AP (temporal-action-detection average precision).

Reference computation:
  - B=256 videos, N=4000 proposals, G=50 ground-truths, IoU thresholds (0.5, 0.75).
  - Per (video, thr): pot[n,g] = IoU(seg_n, gt_g) > thr; greedy matching over
    GT columns claims the first (lowest-index) unused candidate -> is_TP[B,N].
  - Global: sort all B*N scores desc, cumsum TP, AP = sum |dx| * cummax(y).

Uses the identity  IoU > tau  <=>  la + lb - kinv*u > 0  with
kinv = (1+tau)/(1-tau) and u = |as-bs| + |ae-be|.

Device part (8 NeuronCores, data-parallel over B; 32 videos/core,
processed as 16 video pairs):
  - TensorE broadcasts each pair's as/ae rows (exact two-term bf16 splits,
    so bf16-rate matmuls reproduce fp32 values exactly in PSUM) across the
    100 (video-in-pair, GT) partitions.
  - ScalarE computes t1=|as-bs|, t2=|ae-be| via Abs activation with the
    per-(video,GT) boundary as bias, writing bf16.
  - VectorE sums u = t1 + t2 in bf16 (2x DVE mode), DMA'd out per pair.
Host part: thresholds the exported u against both IoU cutoffs (exact fp32
la/lb recomputed from the inputs), runs the exact greedy matching per
(video, thr) via vectorized argmax rounds, then the global ranking of TP
confidences (one sort) and the AP sum.
"""

import os
import numpy as np
import ml_dtypes

import concourse.bass as bass
import concourse.tile as tile
from concourse import bacc, mybir
from concourse.bass_utils import run_bass_kernel_spmd

# problem constants (hardcoded per spec nn_AP_19258633355825)
B, N, G = 256, 4000, 50
NCORES = 8
NV = B // NCORES          # videos per core
NP2 = NV // 2             # video pairs per core
NPAD = 4096               # padded proposal dim
HALF = 2048               # processed in halves (PSUM capacity)
KINV = (3.0, 7.0)         # (1+tau)/(1-tau) for tau in (0.5, 0.75)
F32 = mybir.dt.float32
BF16 = mybir.dt.bfloat16
NPBF = ml_dtypes.bfloat16
PAD_VAL = 1.0e6           # sentinel start/end for padded proposals


def _split2(x):
    """Exact 2-term bf16 split: x ~ h1 + h2 with |err| <= 2^-18 |x|."""
    h1 = x.astype(NPBF)
    h2 = (x - h1.astype(np.float32)).astype(NPBF)
    return h1, h2


# ----------------------------------------------------------------- constants
def _core_inputs(seg, lab):
    """Host-side preprocessing for one core's shard (seg [NV,N,2], lab [NV,G,2])."""
    as_ = np.full((NV, NPAD), PAD_VAL, np.float32)
    ae = np.full((NV, NPAD), PAD_VAL, np.float32)
    as_[:, :N] = seg[:, :, 0]
    ae[:, :N] = seg[:, :, 1]

    sg4 = np.empty((NV, 4, NPAD), NPBF)
    sg4[:, 0], sg4[:, 1] = _split2(as_)
    sg4[:, 2], sg4[:, 3] = _split2(ae)

    # o8v [NP2, 10, 200]: broadcast lhsT per pair. stg rows: (video r, kind)
    # with kinds [as_h1, as_h2, ae_h1, ae_h2], rows 8/9 = ones. Columns
    # [0:100] -> as - bs, [100:200] -> ae - be; within, m = r*50 + g (the GT
    # boundary enters via exact bf16-split coefficients on the ones rows).
    o8v = np.zeros((NP2, 10, 200), np.float32)
    bh1, bh2 = _split2(lab[:, :, 0])     # [NV, G]
    eh1, eh2 = _split2(lab[:, :, 1])
    for r in range(2):
        o8v[:, 4 * r + 0, r * 50:r * 50 + 50] = 1.0        # as_h1
        o8v[:, 4 * r + 1, r * 50:r * 50 + 50] = 1.0        # as_h2
        o8v[:, 4 * r + 2, 100 + r * 50:100 + r * 50 + 50] = 1.0  # ae_h1
        o8v[:, 4 * r + 3, 100 + r * 50:100 + r * 50 + 50] = 1.0  # ae_h2
        o8v[:, 8, r * 50:r * 50 + 50] = -bh1[r::2].astype(np.float32)
        o8v[:, 9, r * 50:r * 50 + 50] = -bh2[r::2].astype(np.float32)
        o8v[:, 8, 100 + r * 50:100 + r * 50 + 50] = -eh1[r::2].astype(np.float32)
        o8v[:, 9, 100 + r * 50:100 + r * 50 + 50] = -eh2[r::2].astype(np.float32)
    ones = np.ones((2, NPAD), NPBF)
    return {"sg4": sg4, "o8v": o8v.astype(NPBF), "ones": ones}


# ----------------------------------------------------------------- device IR
def build_nc():
    nc = bacc.Bacc("TRN2", target_bir_lowering=False, debug=False,
                   num_devices=NCORES)

    sg4_d = nc.dram_tensor("sg4", [NV, 4, NPAD], BF16, kind="ExternalInput")
    o8v_d = nc.dram_tensor("o8v", [NP2, 10, 200], BF16, kind="ExternalInput")
    ones_d = nc.dram_tensor("ones", [2, NPAD], BF16, kind="ExternalInput")
    out = nc.dram_tensor("out", [100, NP2 * NPAD], BF16, kind="ExternalOutput")

    with tile.TileContext(nc) as tc:
        with (
            tc.tile_pool(name="stg", bufs=3) as stgp,
            tc.tile_pool(name="lhp", bufs=3) as lhp,
            tc.tile_pool(name="t12", bufs=2) as t12p,
            tc.tile_pool(name="u", bufs=3) as up,
            tc.tile_pool(name="ps_a", bufs=2, space="PSUM") as ps_a,
        ):
            for p in range(NP2):
                vA = 2 * p
                # staging: [10, NPAD] = both videos' 4 split rows + ones
                stg = stgp.tile([10, NPAD], BF16)
                nc.sync.dma_start(
                    stg[0:8], sg4_d[vA:vA + 2].rearrange("v k n -> (v k) n"))
                nc.sync.dma_start(stg[8:10], ones_d[:])
                o8v = lhp.tile([10, 200], BF16)
                nc.sync.dma_start(o8v[:], o8v_d[p])

                t1 = t12p.tile([100, NPAD], BF16, tag="t1")
                t2 = t12p.tile([100, NPAD], F32, tag="t2")
                for part in range(2):      # 0: |as-bs| -> t1, 1: |ae-be| -> t2
                    tdst = t1 if part == 0 else t2
                    lhs = o8v[:, part * 100:part * 100 + 100]
                    for h in range(2):
                        hw = HALF if h == 0 else N - HALF
                        aps = ps_a.tile([100, HALF], F32)
                        hs = slice(h * HALF, h * HALF + hw)
                        for c in range((hw + 511) // 512):
                            w = min(512, hw - c * 512)
                            ns = slice(h * HALF + c * 512,
                                       h * HALF + c * 512 + w)
                            ms = slice(c * 512, c * 512 + w)
                            nc.tensor.matmul(aps[:, ms], lhs, stg[:, ns],
                                             start=True, stop=True)
                        # abs: balance across ScalarE (Abs activation) and
                        # VectorE (clear the fp32 sign bit via bitwise_and)
                        on_dve = part == 1 and (h == 1 or p % 2 == 0)
                        if on_dve:
                            nc.vector.tensor_scalar(
                                tdst[:, hs].bitcast(mybir.dt.uint32),
                                aps[:, 0:hw].bitcast(mybir.dt.uint32),
                                0x7FFFFFFF, None,
                                mybir.AluOpType.bitwise_and)
                        else:
                            nc.scalar.activation(
                                tdst[:, hs], aps[:, 0:hw],
                                mybir.ActivationFunctionType.Abs)

                u = up.tile([100, NPAD], BF16)
                # u = t1 + t2, split across VectorE / GpSimd
                nc.vector.tensor_tensor(u[:, 0:HALF], t1[:, 0:HALF],
                                        t2[:, 0:HALF], mybir.AluOpType.add)
                nc.gpsimd.tensor_tensor(u[:, HALF:N], t1[:, HALF:N],
                                        t2[:, HALF:N], mybir.AluOpType.add)
                nc.sync.dma_start(out[:, p * NPAD:p * NPAD + N], u[:, 0:N])
    nc.compile()
    return nc


_NC_CACHE = None


def _get_nc():
    global _NC_CACHE
    if _NC_CACHE is None:
        _NC_CACHE = build_nc()
    return _NC_CACHE


# ------------------------------------------------------------------ host post
def _greedy_from_u(u, segments, labels):
    """u [B, 100, N] f32 (rows r*50+g within each pair -> here already
    re-indexed to [B, G, N]); exact greedy per (video, thr).
    Returns is_tp [2, B, N] bool."""
    la = (segments[:, :, 1] - segments[:, :, 0]).astype(np.float32)  # [B, N]
    lb = (labels[:, :, 1] - labels[:, :, 0]).astype(np.float32)      # [B, G]
    is_tp = np.empty((2, B, N), bool)
    rows = np.arange(B)
    for t in range(2):
        kinv = np.float32(KINV[t])
        used = np.zeros((B, N), bool)
        for g in range(G):
            margin = la + lb[:, g:g + 1] - kinv * u[:, g]            # [B, N]
            cand = (margin > 0) & ~used
            idx = np.argmax(cand, axis=1)
            has = np.take_along_axis(cand, idx[:, None], axis=1)[:, 0]
            used[rows[has], idx[has]] = True
        is_tp[t] = used
    return is_tp


def _ap_from_tp(is_tp, scores):
    """is_tp [2, B, N] bool, scores [B, N] -> AP [2] float32 (exact ranking)."""
    conf = scores.reshape(-1)
    M = conf.size
    bits = conf.view(np.uint32).astype(np.int64)
    key = (bits << 20) + (2**20 - 1 - np.arange(M, dtype=np.int64))
    skey = np.sort(key)
    out = np.empty(2, np.float32)
    for t in range(2):
        tp_idx = np.nonzero(is_tp[t].reshape(-1))[0]
        k = key[tp_idx]
        # rank (1-based) in descending order = #{keys > k} + 1
        r = np.sort(M - np.searchsorted(skey, k, side="left"))
        kk = np.arange(1, len(r) + 1, dtype=np.float64)
        prec = (kk / r).astype(np.float32)
        sufmax = np.maximum.accumulate(prec[::-1])[::-1]
        out[t] = np.float32(sufmax.astype(np.float64).sum() / (B * G))
    return out


def _enable_profiling():
    """Dev-only: register the NTFF profiling hook (missing antenv shim) and
    keep artifacts local. Returns extra kwargs for run_bass_kernel_spmd."""
    import sys
    import types
    import tempfile

    if "antenv.axon_hooks" not in sys.modules:
        mod = types.ModuleType("antenv.axon_hooks")
        _h = [None]
        mod.set_axon_ntff_profile_hook = lambda h: _h.__setitem__(0, h)
        mod.get_axon_ntff_profile_hook = lambda: _h[0]
        sys.modules["antenv.axon_hooks"] = mod
        from trn_agent_boot.trn_boot import _ntff_profile_via_ctypes
        mod.set_axon_ntff_profile_hook(
            _ntff_profile_via_ctypes("/opt/axon/libaxon_pjrt.so"))
    import concourse.bass_utils as bu
    bu.upload_artifacts = lambda tmpdir: tmpdir
    tdir = os.environ.get("ATH_TRACE_DIR") or tempfile.mkdtemp(
        prefix="ap_trace_")
    print("trace dir:", tdir)
    return {"tmpdir": tdir}


# ------------------------------------------------------------------- kernel
def kernel(scores, segments, labels):
    scores = np.ascontiguousarray(scores, np.float32)
    segments = np.ascontiguousarray(segments, np.float32)
    labels = np.ascontiguousarray(labels, np.float32)

    in_maps = []
    for i in range(NCORES):
        sl = slice(i * NV, (i + 1) * NV)
        in_maps.append(_core_inputs(segments[sl], labels[sl]))
    nc = _get_nc()
    trace = bool(int(os.environ.get("ATH_PROFILE", "0")))
    kw = {}
    if trace:
        try:
            kw = _enable_profiling()
        except Exception as e:           # profiling is best-effort
            print("profiling unavailable:", e)
            trace = False
    res = run_bass_kernel_spmd(nc, in_maps, core_ids=list(range(NCORES)),
                               trace=trace, **kw)
    if trace and res.exec_time_ns is not None:
        print(f"HW exec time: {res.exec_time_ns} ns")

    # dev out: [100, NP2*NPAD] bf16 per core, rows (r*50+g), col p*NPAD + n
    u = np.empty((B, G, N), np.float32)
    for i in range(NCORES):
        d = np.asarray(res.results[i]["out"]).astype(np.float32)
        d = d.reshape(2, G, NP2, NPAD)           # [r, g, p, n]
        for r in range(2):
            u[i * NV + 2 * np.arange(NP2) + r] = \
                d[r].transpose(1, 0, 2)[:, :, :N]

    is_tp = _greedy_from_u(u, segments, labels)
    return _ap_from_tp(is_tp, scores)


# revision 22
# speedup vs baseline: 1.7214x; 1.7214x over previous
"""Trainium2 kernel for nn_AP (temporal-action-detection average precision).

Reference computation:
  - B=256 videos, N=4000 proposals, G=50 ground-truths, IoU thresholds (0.5, 0.75).
  - Per (video, thr): pot[n,g] = IoU(seg_n, gt_g) > thr; greedy matching over
    GT columns claims the first (lowest-index) unused candidate -> is_TP[B,N].
  - Global: sort all B*N scores desc, cumsum TP, AP = sum |dx| * cummax(y).

Uses the identity  IoU > tau  <=>  la + lb - kinv*u > 0  with
kinv = (1+tau)/(1-tau) and u = |as-bs| + |ae-be|.

Device part (8 NeuronCores, data-parallel over B; 32 videos/core,
processed as 16 video pairs):
  - TensorE broadcasts each pair's as/ae rows (exact two-term bf16 splits,
    so bf16-rate matmuls reproduce fp32 values exactly in PSUM) across the
    100 (video-in-pair, GT) partitions.
  - ScalarE computes t1=|as-bs|, t2=|ae-be| via Abs activation with the
    per-(video,GT) boundary as bias, writing bf16.
  - VectorE sums u = t1 + t2 in bf16 (2x DVE mode), DMA'd out per pair.
Host part: thresholds the exported u against both IoU cutoffs (exact fp32
la/lb recomputed from the inputs), runs the exact greedy matching per
(video, thr) via vectorized argmax rounds, then the global ranking of TP
confidences (one sort) and the AP sum.
"""

import os
import numpy as np
import ml_dtypes

import concourse.bass as bass
import concourse.tile as tile
from concourse import bacc, mybir
from concourse.bass_utils import run_bass_kernel_spmd

# problem constants (hardcoded per spec nn_AP_19258633355825)
B, N, G = 256, 4000, 50
NCORES = 8
NV = B // NCORES          # videos per core
NP2 = NV // 2             # video pairs per core
NPAD = 4096               # padded proposal dim
HALF = 2048               # processed in halves (PSUM capacity)
KINV = (3.0, 7.0)         # (1+tau)/(1-tau) for tau in (0.5, 0.75)
F32 = mybir.dt.float32
BF16 = mybir.dt.bfloat16
NPBF = ml_dtypes.bfloat16
PAD_VAL = 1.0e6           # sentinel start/end for padded proposals


def _split2(x):
    """Exact 2-term bf16 split: x ~ h1 + h2 with |err| <= 2^-18 |x|."""
    h1 = x.astype(NPBF)
    h2 = (x - h1.astype(np.float32)).astype(NPBF)
    return h1, h2


# ----------------------------------------------------------------- constants
def _core_inputs(seg, lab):
    """Host-side preprocessing for one core's shard (seg [NV,N,2], lab [NV,G,2])."""
    as_ = np.full((NV, NPAD), PAD_VAL, np.float32)
    ae = np.full((NV, NPAD), PAD_VAL, np.float32)
    as_[:, :N] = seg[:, :, 0]
    ae[:, :N] = seg[:, :, 1]

    # u = |as-bs| + |ae-be| = max(|P|, |Q|) with P = (as+ae) - (bs+be) and
    # Q = (as-ae) - (bs-be).  stg rows: 4r+0/1 = split(as+ae), 4r+2/3 =
    # split(as-ae) for video-in-pair r; rows 8/9 = ones.  o8v columns
    # [0:100] -> P, [100:200] -> Q; within, m = r*50 + g (the GT terms
    # enter via exact bf16-split coefficients on the ones rows).
    sgp = np.ones((NP2, 10, NPAD), NPBF)
    sh1, sh2 = _split2(as_ + ae)
    dh1, dh2 = _split2(as_ - ae)
    o8v = np.zeros((NP2, 10, 200), np.float32)
    bsum1, bsum2 = _split2(lab[:, :, 0] + lab[:, :, 1])   # [NV, G]
    bdif1, bdif2 = _split2(lab[:, :, 0] - lab[:, :, 1])
    for r in range(2):
        sgp[:, 4 * r + 0] = sh1[r::2]
        sgp[:, 4 * r + 1] = sh2[r::2]
        sgp[:, 4 * r + 2] = dh1[r::2]
        sgp[:, 4 * r + 3] = dh2[r::2]
        o8v[:, 4 * r + 0, r * 50:r * 50 + 50] = 1.0
        o8v[:, 4 * r + 1, r * 50:r * 50 + 50] = 1.0
        o8v[:, 4 * r + 2, 100 + r * 50:100 + r * 50 + 50] = 1.0
        o8v[:, 4 * r + 3, 100 + r * 50:100 + r * 50 + 50] = 1.0
        o8v[:, 8, r * 50:r * 50 + 50] = -bsum1[r::2].astype(np.float32)
        o8v[:, 9, r * 50:r * 50 + 50] = -bsum2[r::2].astype(np.float32)
        o8v[:, 8, 100 + r * 50:100 + r * 50 + 50] = -bdif1[r::2].astype(np.float32)
        o8v[:, 9, 100 + r * 50:100 + r * 50 + 50] = -bdif2[r::2].astype(np.float32)
    return {"sgp": sgp, "o8v": o8v.astype(NPBF)}


# ----------------------------------------------------------------- device IR
def build_nc():
    nc = bacc.Bacc("TRN2", target_bir_lowering=False, debug=False,
                   num_devices=NCORES)

    sgp_d = nc.dram_tensor("sgp", [NP2, 10, NPAD], BF16, kind="ExternalInput")
    o8v_d = nc.dram_tensor("o8v", [NP2, 10, 200], BF16, kind="ExternalInput")
    out = nc.dram_tensor("out", [100, NP2 * NPAD], BF16, kind="ExternalOutput")

    with tile.TileContext(nc) as tc:
        with (
            tc.tile_pool(name="stg", bufs=3) as stgp,
            tc.tile_pool(name="lhp", bufs=3) as lhp,
            tc.tile_pool(name="t12", bufs=2) as t12p,
            tc.tile_pool(name="u", bufs=3) as up,
            tc.tile_pool(name="ps_q", bufs=2, space="PSUM") as ps_q,
            tc.tile_pool(name="ps_p", bufs=2, space="PSUM") as ps_p,
        ):
            for p in range(NP2):
                stg = stgp.tile([10, NPAD], BF16)
                nc.sync.dma_start(stg[:], sgp_d[p])
                o8v = lhp.tile([10, 200], BF16)
                nc.sync.dma_start(o8v[:], o8v_d[p])

                tq = t12p.tile([100, NPAD], BF16, tag="tq")
                tp = t12p.tile([100, NPAD], BF16, tag="tp")
                sm = t12p.tile([100, NPAD], BF16, tag="sm")
                u = up.tile([100, NPAD], BF16)
                QW = 1024
                for h in range(4):
                    hw = QW if h < 3 else N - 3 * QW
                    hs = slice(h * QW, h * QW + hw)
                    # Q = (as-ae) - (bs-be): ScalarE |Q| -> tq
                    apq = ps_q.tile([100, QW], F32)
                    for c in range((hw + 511) // 512):
                        w = min(512, hw - c * 512)
                        ns = slice(h * QW + c * 512, h * QW + c * 512 + w)
                        nc.tensor.matmul(apq[:, c * 512:c * 512 + w],
                                         o8v[:, 100:200], stg[:, ns],
                                         start=True, stop=True)
                    nc.scalar.activation(tq[:, hs], apq[:, 0:hw],
                                         mybir.ActivationFunctionType.Abs)
                    # P = (as+ae) - (bs+be): u = max(|P|, |Q|)
                    app = ps_p.tile([100, QW], F32)
                    for c in range((hw + 511) // 512):
                        w = min(512, hw - c * 512)
                        ns = slice(h * QW + c * 512, h * QW + c * 512 + w)
                        nc.tensor.matmul(app[:, c * 512:c * 512 + w],
                                         o8v[:, 0:100], stg[:, ns],
                                         start=True, stop=True)
                    qi = p * 4 + h
                    if qi % 2 == 0:
                        # path B: ScalarE |P|, VectorE bf16 max (2x mode)
                        nc.scalar.activation(tp[:, hs], app[:, 0:hw],
                                             mybir.ActivationFunctionType.Abs)
                        nc.vector.tensor_tensor(u[:, hs], tp[:, hs],
                                                tq[:, hs],
                                                mybir.AluOpType.max)
                    else:
                        # path C: two VectorE max ops against the PSUM P
                        nc.vector.scalar_tensor_tensor(
                            sm[:, hs], app[:, 0:hw], 1.0, tq[:, hs],
                            mybir.AluOpType.mult, mybir.AluOpType.max)
                        nc.vector.scalar_tensor_tensor(
                            u[:, hs], app[:, 0:hw], -1.0, sm[:, hs],
                            mybir.AluOpType.mult, mybir.AluOpType.max)
                nc.sync.dma_start(out[:, p * NPAD:p * NPAD + N], u[:, 0:N])
    nc.compile()
    return nc


_NC_CACHE = None


def _get_nc():
    global _NC_CACHE
    if _NC_CACHE is None:
        _NC_CACHE = build_nc()
    return _NC_CACHE


# ------------------------------------------------------------------ host post
def _greedy_from_u(u, segments, labels):
    """u [B, 100, N] f32 (rows r*50+g within each pair -> here already
    re-indexed to [B, G, N]); exact greedy per (video, thr).
    Returns is_tp [2, B, N] bool."""
    la = (segments[:, :, 1] - segments[:, :, 0]).astype(np.float32)  # [B, N]
    lb = (labels[:, :, 1] - labels[:, :, 0]).astype(np.float32)      # [B, G]
    is_tp = np.empty((2, B, N), bool)
    rows = np.arange(B)
    for t in range(2):
        kinv = np.float32(KINV[t])
        used = np.zeros((B, N), bool)
        for g in range(G):
            margin = la + lb[:, g:g + 1] - kinv * u[:, g]            # [B, N]
            cand = (margin > 0) & ~used
            idx = np.argmax(cand, axis=1)
            has = np.take_along_axis(cand, idx[:, None], axis=1)[:, 0]
            used[rows[has], idx[has]] = True
        is_tp[t] = used
    return is_tp


def _ap_from_tp(is_tp, scores):
    """is_tp [2, B, N] bool, scores [B, N] -> AP [2] float32 (exact ranking)."""
    conf = scores.reshape(-1)
    M = conf.size
    bits = conf.view(np.uint32).astype(np.int64)
    key = (bits << 20) + (2**20 - 1 - np.arange(M, dtype=np.int64))
    skey = np.sort(key)
    out = np.empty(2, np.float32)
    for t in range(2):
        tp_idx = np.nonzero(is_tp[t].reshape(-1))[0]
        k = key[tp_idx]
        # rank (1-based) in descending order = #{keys > k} + 1
        r = np.sort(M - np.searchsorted(skey, k, side="left"))
        kk = np.arange(1, len(r) + 1, dtype=np.float64)
        prec = (kk / r).astype(np.float32)
        sufmax = np.maximum.accumulate(prec[::-1])[::-1]
        out[t] = np.float32(sufmax.astype(np.float64).sum() / (B * G))
    return out


def _enable_profiling():
    """Dev-only: register the NTFF profiling hook (missing antenv shim) and
    keep artifacts local. Returns extra kwargs for run_bass_kernel_spmd."""
    import sys
    import types
    import tempfile

    if "antenv.axon_hooks" not in sys.modules:
        mod = types.ModuleType("antenv.axon_hooks")
        _h = [None]
        mod.set_axon_ntff_profile_hook = lambda h: _h.__setitem__(0, h)
        mod.get_axon_ntff_profile_hook = lambda: _h[0]
        sys.modules["antenv.axon_hooks"] = mod
        from trn_agent_boot.trn_boot import _ntff_profile_via_ctypes
        mod.set_axon_ntff_profile_hook(
            _ntff_profile_via_ctypes("/opt/axon/libaxon_pjrt.so"))
    import concourse.bass_utils as bu
    bu.upload_artifacts = lambda tmpdir: tmpdir
    tdir = os.environ.get("ATH_TRACE_DIR") or tempfile.mkdtemp(
        prefix="ap_trace_")
    print("trace dir:", tdir)
    return {"tmpdir": tdir}


# ------------------------------------------------------------------- kernel
def kernel(scores, segments, labels):
    scores = np.ascontiguousarray(scores, np.float32)
    segments = np.ascontiguousarray(segments, np.float32)
    labels = np.ascontiguousarray(labels, np.float32)

    in_maps = []
    for i in range(NCORES):
        sl = slice(i * NV, (i + 1) * NV)
        in_maps.append(_core_inputs(segments[sl], labels[sl]))
    nc = _get_nc()
    trace = bool(int(os.environ.get("ATH_PROFILE", "0")))
    kw = {}
    if trace:
        try:
            kw = _enable_profiling()
        except Exception as e:           # profiling is best-effort
            print("profiling unavailable:", e)
            trace = False
    res = run_bass_kernel_spmd(nc, in_maps, core_ids=list(range(NCORES)),
                               trace=trace, **kw)
    if trace and res.exec_time_ns is not None:
        print(f"HW exec time: {res.exec_time_ns} ns")

    # dev out: [100, NP2*NPAD] bf16 per core, rows (r*50+g), col p*NPAD + n
    u = np.empty((B, G, N), np.float32)
    for i in range(NCORES):
        d = np.asarray(res.results[i]["out"]).astype(np.float32)
        d = d.reshape(2, G, NP2, NPAD)           # [r, g, p, n]
        for r in range(2):
            u[i * NV + 2 * np.arange(NP2) + r] = \
                d[r].transpose(1, 0, 2)[:, :, :N]

    is_tp = _greedy_from_u(u, segments, labels)
    return _ap_from_tp(is_tp, scores)
